# revision 5
# baseline (speedup 1.0000x reference)
"""Trainium2 Bass kernel for CustomPatchEmbedding (ragged patch gather + two projections).

Strategy (data-parallel over batch, 8 cores x 4 images):
  - Images are repacked on host into a sliding 8-row-block channel-last
    layout: blk8[y, x, dy, c] = img[c, y+dy, x] (8x redundant, bf16). A fine
    16x16 patch is then TWO contiguous 384-element runs (blocks y and y+8); a
    coarse 64x64 patch is EIGHT contiguous 1536-element runs. The HW indirect
    DMA consumes exactly one offset per destination partition, so each gather
    instruction moves 128 patch-runs; the whole gather is 24 instructions
    (the naive per-row gather needs 576, at ~1us of GpSimd SWDGE time each).
  - Weights are cast to bf16 and column-permuted on host to match the
    (block, dx, dy, c) feature order; PSUM accumulates fp32.
  - TensorE transposes 128-feature chunks to [feature, patch] (4 chunks per
    PSUM bank, one DVE copy per bank), then accumulates bf16 matmuls into
    PSUM [patch, 256] fp32.
  - Fine weights are resident in SBUF; coarse weights stream as 12 large
    [128, 2048] DMAs.

kernel(**inputs) takes the FULL unsharded inputs and returns (32, 288, 256) f32.
"""
import sys
import numpy as np

sys.path.insert(0, "/opt/trn_rl_repo")

import ml_dtypes
import concourse.bass as bass
import concourse.bacc as bacc
import concourse.mybir as mybir
import concourse.tile as tile
from concourse.masks import make_identity
from concourse.bass_utils import run_bass_kernel_spmd
from contextlib import ExitStack

# Problem constants (hardcoded per spec).
B, C, H, W = 32, 3, 512, 512
FP, CP = 16, 64
NF, NCO = 256, 32
D = 256
NCORES = 8
IPC = B // NCORES              # images per core
CHW = C * H * W                # 786432, per-image flat element count
KF = C * FP * FP               # 768  fine features
KC = C * CP * CP               # 12288 coarse features
P = 128
GF = IPC * 2                   # fine groups of 128 patches per core
NKF = KF // P                  # 6 fine k-chunks
NKC = KC // P                  # 96 coarse k-chunks

R = 8                          # rows packed per block in the sliding layout
RUN_F = FP * R * C             # 384  elements per fine gather run
RUN_C = CP * R * C             # 1536 elements per coarse gather run
NBF = FP // R                  # 2 runs (vertical blocks) per fine patch
NBC = CP // R                  # 8 runs per coarse patch
BLK_STRIDE = W * R * C         # 12288 elements per block row
IMG8 = H * BLK_STRIDE          # padded per-image element count (blocks 0..511)
NFLAT8 = IPC * IMG8

FDT = mybir.dt.float32
BDT = mybir.dt.bfloat16
IDT = mybir.dt.int32
BF16 = ml_dtypes.bfloat16


def _emit(nc, tc, t):
    """Emit the per-core Tile program. `t` maps tensor name -> dram handle."""
    with ExitStack() as ctx:
        const = ctx.enter_context(tc.tile_pool(name="const", bufs=1))
        gf_pool = ctx.enter_context(tc.tile_pool(name="gf", bufs=3))
        gc_pool = ctx.enter_context(tc.tile_pool(name="gc", bufs=2))
        wc_pool = ctx.enter_context(tc.tile_pool(name="wc", bufs=3))
        lt4_pool = ctx.enter_context(tc.tile_pool(name="lt4", bufs=4))
        lt2_pool = ctx.enter_context(tc.tile_pool(name="lt2", bufs=2))
        ob_pool = ctx.enter_context(tc.tile_pool(name="ob", bufs=3))
        ps_tp = ctx.enter_context(tc.tile_pool(name="ps_tp", bufs=3, space="PSUM"))
        ps_f = ctx.enter_context(tc.tile_pool(name="ps_f", bufs=2, space="PSUM"))
        ps_c = ctx.enter_context(tc.tile_pool(name="ps_c", bufs=1, space="PSUM"))

        # --- constants ---
        identity = const.tile([P, P], BDT)
        make_identity(nc, identity[:])
        bias_f = const.tile([P, D], FDT)
        nc.sync.dma_start(bias_f[:], t["bias_f"][:])
        bias_c = const.tile([P, D], FDT)
        nc.sync.dma_start(bias_c[:], t["bias_c"][:])
        # fine weights resident: [128, 6*256], chunk k at cols [k*256, (k+1)*256)
        wf = const.tile([P, NKF * D], BDT)
        nc.sync.dma_start(wf[:], t["wf2"][:])
        # gather offset tiles (host-precomputed element offsets into imgs8)
        fidx = const.tile([P, GF * NBF], IDT)
        nc.sync.dma_start(fidx[:], t["fidx"][:])
        cidx = const.tile([P, NBC], IDT)
        nc.sync.dma_start(cidx[:], t["cidx"][:])

        imgs8 = t["imgs8"]
        out = t["out"]

        def mm_block(gt, col0, psum, wsrc, wcol0, kglobal0, nk, kstart0, kstop):
            """Transpose nk 128-col chunks of gt (from col0) and matmul into psum."""
            for st in range(0, nk, 4):
                cnt = min(4, nk - st)
                tp = ps_tp.tile([P, 512], BDT, tag="tp")
                for i in range(cnt):
                    nc.tensor.transpose(
                        out=tp[:, i * P:(i + 1) * P],
                        in_=gt[:, col0 + (st + i) * P:col0 + (st + i + 1) * P],
                        identity=identity[:],
                    )
                if cnt == 4:
                    lt = lt4_pool.tile([P, 512], BDT, tag="lt4")
                    nc.vector.tensor_copy(lt[:], tp[:])
                else:
                    lt = lt2_pool.tile([P, cnt * P], BDT, tag="lt2")
                    nc.vector.tensor_copy(lt[:], tp[:, 0:cnt * P])
                for i in range(cnt):
                    k = kglobal0 + st + i
                    nc.tensor.matmul(
                        out=psum[:], lhsT=lt[:, i * P:(i + 1) * P],
                        rhs=wsrc[:, wcol0 + (st + i) * D:wcol0 + (st + i + 1) * D],
                        start=(k == kstart0), stop=(k == kstop),
                    )

        # --- fine branch: 8 groups of 128 patches, 2 gather runs per patch ---
        for g in range(GF):
            b, hh = divmod(g, 2)
            gt = gf_pool.tile([P, KF], BDT)
            for kb in range(NBF):
                nc.gpsimd.indirect_dma_start(
                    out=gt[:, kb * RUN_F:(kb + 1) * RUN_F], out_offset=None,
                    in_=imgs8[:],
                    in_offset=bass.IndirectOffsetOnAxis(
                        ap=fidx[:, g * NBF + kb:g * NBF + kb + 1], axis=0
                    ),
                )
            psum = ps_f.tile([P, D], FDT)
            mm_block(gt, 0, psum, wf, 0, 0, NKF, 0, NKF - 1)
            ob = ob_pool.tile([P, D], FDT, tag="ob")
            nc.vector.tensor_tensor(
                out=ob[:], in0=psum[:], in1=bias_f[:], op=mybir.AluOpType.add
            )
            row0 = b * (NF + NCO) + hh * P
            nc.scalar.dma_start(out[row0:row0 + P, :], ob[:])

        # --- coarse branch: 2 half-tiles of 4 runs each, 96 k-chunks total ---
        psum_c = ps_c.tile([P, D], FDT)
        KHALF = NKC // 2                      # 48 chunks per half
        for half in range(2):
            gt = gc_pool.tile([P, 4 * RUN_C], BDT)
            for kbl in range(4):
                kb = half * 4 + kbl
                nc.gpsimd.indirect_dma_start(
                    out=gt[:, kbl * RUN_C:(kbl + 1) * RUN_C], out_offset=None,
                    in_=imgs8[:],
                    in_offset=bass.IndirectOffsetOnAxis(
                        ap=cidx[:, kb:kb + 1], axis=0
                    ),
                )
            for sc in range(6):
                s = half * 6 + sc
                wc = wc_pool.tile([P, 8 * D], BDT)
                nc.sync.dma_start(wc[:], t["wc2"][:, s * 8 * D:(s + 1) * 8 * D])
                mm_block(gt, sc * 8 * P, psum_c, wc, 0,
                         half * KHALF + sc * 8, 8, 0, NKC - 1)
        oc = ob_pool.tile([P, D], FDT, tag="oc")
        nc.vector.tensor_tensor(
            out=oc[:], in0=psum_c[:], in1=bias_c[:], op=mybir.AluOpType.add
        )
        for b in range(IPC):
            nc.scalar.dma_start(
                out[b * (NF + NCO) + NF:b * (NF + NCO) + NF + NCO, :],
                oc[b * NCO:(b + 1) * NCO, :],
            )


def build(reps: int = 1):
    nc = bacc.Bacc("TRN2", target_bir_lowering=False, debug=False)
    t = {
        "imgs8": nc.dram_tensor("imgs8", [NFLAT8, 1], BDT, kind="ExternalInput"),
        "wf2": nc.dram_tensor("wf2", [P, NKF * D], BDT, kind="ExternalInput"),
        "wc2": nc.dram_tensor("wc2", [P, NKC * D], BDT, kind="ExternalInput"),
        "bias_f": nc.dram_tensor("bias_f", [P, D], FDT, kind="ExternalInput"),
        "bias_c": nc.dram_tensor("bias_c", [P, D], FDT, kind="ExternalInput"),
        "fidx": nc.dram_tensor("fidx", [P, GF * NBF], IDT, kind="ExternalInput"),
        "cidx": nc.dram_tensor("cidx", [P, NBC], IDT, kind="ExternalInput"),
        "out": nc.dram_tensor("out", [IPC * (NF + NCO), D], FDT, kind="ExternalOutput"),
    }
    with tile.TileContext(nc) as tc:
        for _ in range(reps):
            _emit(nc, tc, t)
    nc.compile()
    return nc


def repack_images(images):
    """[B, C, H, W] f32 -> sliding 8-row-block channel-last bf16.

    blk8[b, y, x, dy, c] = images[b, c, y+dy, x]; y padded to H blocks
    (blocks H-R+1..H-1 unused).
    """
    cl = np.ascontiguousarray(images.transpose(0, 2, 3, 1)).astype(BF16)  # [B, y, x, c]
    sw = np.lib.stride_tricks.sliding_window_view(cl, R, axis=1)  # [B, H-R+1, x, c, dy]
    sw = sw.transpose(0, 1, 2, 4, 3)                              # [B, blk, x, dy, c]
    blob = np.zeros((images.shape[0], H, W, R, C), dtype=BF16)
    blob[:, :H - R + 1] = sw
    return blob


def host_indices(fine_xy, coarse_xy):
    """Element offsets into the per-core imgs8 blob (one per gather run)."""
    kb_f = np.arange(NBF) * R                                     # [2]
    base_f = (fine_xy[:, :, 1][..., None] + kb_f) * BLK_STRIDE \
        + fine_xy[:, :, 0][..., None] * (R * C) \
        + (np.arange(IPC) * IMG8)[:, None, None]                  # [IPC, NF, 2]
    fidx = base_f.reshape(GF, P, NBF).transpose(1, 0, 2).reshape(P, GF * NBF)

    kb_c = np.arange(NBC) * R                                     # [8]
    cidx = (coarse_xy[:, :, 1][..., None] + kb_c) * BLK_STRIDE \
        + coarse_xy[:, :, 0][..., None] * (R * C) \
        + (np.arange(IPC) * IMG8)[:, None, None]                  # [IPC, NCO, 8]
    cidx = cidx.reshape(P, NBC)
    return (np.ascontiguousarray(fidx.astype(np.int32)),
            np.ascontiguousarray(cidx.astype(np.int32)))


def feat_perm(patch, nb):
    """New feature order (kb, dx, dy8, c) -> original (c, dy, dx) column index."""
    kb, dx, dy8, c = np.meshgrid(
        np.arange(nb), np.arange(patch), np.arange(R), np.arange(C), indexing="ij"
    )
    dy = kb * R + dy8
    return (c * (patch * patch) + dy * patch + dx).reshape(-1)


def swizzle_w(wT, perm):
    """[K, D] feature-major weight -> [128, (K//128)*D], permuted to gather order."""
    w = wT[perm]                                                  # [K, D] new order
    K = w.shape[0]
    return np.ascontiguousarray(
        w.reshape(K // P, P, D).transpose(1, 0, 2).reshape(P, (K // P) * D)
    )


def make_in_maps(images, W_fine, b_fine, W_coarse, b_coarse, fine_xy, coarse_xy):
    images = np.asarray(images, dtype=np.float32)
    fine_xy = np.asarray(fine_xy, dtype=np.int64)
    coarse_xy = np.asarray(coarse_xy, dtype=np.int64)
    blob = repack_images(images)
    wf2 = swizzle_w(np.asarray(W_fine, np.float32).T.astype(BF16), feat_perm(FP, NBF))
    wc2 = swizzle_w(np.asarray(W_coarse, np.float32).T.astype(BF16), feat_perm(CP, NBC))
    bias_f = np.ascontiguousarray(np.repeat(np.asarray(b_fine, np.float32)[None, :], P, axis=0))
    bias_c = np.ascontiguousarray(np.repeat(np.asarray(b_coarse, np.float32)[None, :], P, axis=0))
    in_maps = []
    for c in range(NCORES):
        sl = slice(c * IPC, (c + 1) * IPC)
        fidx, cidx = host_indices(fine_xy[sl], coarse_xy[sl])
        in_maps.append({
            "imgs8": blob[sl].reshape(NFLAT8, 1),
            "wf2": wf2, "wc2": wc2,
            "bias_f": bias_f, "bias_c": bias_c,
            "fidx": fidx, "cidx": cidx,
        })
    return in_maps


_NC_CACHE = []


def _get_nc():
    if not _NC_CACHE:
        _NC_CACHE.append(build())
    return _NC_CACHE[0]


def run(inputs: dict, trace: bool = False):
    nc = _get_nc()
    in_maps = make_in_maps(**inputs)
    res = run_bass_kernel_spmd(nc, in_maps, list(range(NCORES)), trace=trace)
    outs = [
        np.asarray(res.results[c]["out"]).reshape(IPC, NF + NCO, D)
        for c in range(NCORES)
    ]
    return np.concatenate(outs, axis=0), res


def kernel(**inputs) -> np.ndarray:
    out, _ = run(inputs, trace=False)
    return out
